# revision 9
# baseline (speedup 1.0000x reference)
"""Trainium2 Bass kernel for nn_IntShear4.

Computes M = S_11 @ S_10 @ ... @ S_0 where
  S_p = I + k_int[p] * e_{i_p} e_{j_p}^T,
  k_int = round_to_nearest_even(3 * tanh(k_raw))       (4x4 matrices)

Device algorithm (single NeuronCore, replicated SPMD on 8 cores):
  - Inputs are DMA-broadcast into SBUF: k_raw as [4, 48] (each of the 12
    values replicated over 4 partitions x 4 free slots), pairs_i/pairs_j
    likewise.  This sidesteps any on-chip partition broadcast.
  - k_int on the Scalar engine: tanh, *3, then the +/- 1.5*2^23
    magic-number trick for exact round-to-nearest-even in f32.
  - One-hot column masks IM/JM [4,48] built with iota + is_equal on the
    Vector engine; k folded into JM.
  - The chain runs entirely on the Vector engine, maintaining R (starting
    at I) and applying R <- R @ S_p for p = 11..0, which is the column op
      R[:, j_p] += k_int[p] * R[:, i_p]
    done index-free with two fused ops per step:
      u = tensor_tensor_reduce(R * IM_p)   # extracts column i_p
      R = scalar_tensor_tensor(KJM_p * u + R)
  - Final R == M, DMA'd straight out.
"""

import numpy as np

import concourse.bacc as bacc
import concourse.bass as bass
import concourse.mybir as mybir
import concourse.tile as tile
from concourse.bass_utils import run_bass_kernel_spmd

F32 = mybir.dt.float32
I32 = mybir.dt.int32
AF = mybir.ActivationFunctionType
ALU = mybir.AluOpType

N_PAIRS = 12
DIM = 4
N_CORES = 8
ROUND_MAGIC = 12582912.0  # 1.5 * 2**23: (x + C) - C == rne_round(x) for |x| < 2**22


def _build_module() -> bass.Bass:
    nc = bacc.Bacc("TRN2", target_bir_lowering=False, debug=False)

    k_raw = nc.dram_tensor("k_raw", [N_PAIRS], F32, kind="ExternalInput")
    pairs_i = nc.dram_tensor("pairs_i", [N_PAIRS], I32, kind="ExternalInput")
    pairs_j = nc.dram_tensor("pairs_j", [N_PAIRS], I32, kind="ExternalInput")
    out = nc.dram_tensor("out", [DIM, DIM], F32, kind="ExternalOutput")

    W = N_PAIRS * DIM  # 48

    with tile.TileContext(nc) as tc:
        with tc.tile_pool(name="pool", bufs=1) as pool:
            kr = pool.tile([DIM, W], F32, tag="kr")
            irep = pool.tile([DIM, W], I32, tag="irep")
            jrep = pool.tile([DIM, W], I32, tag="jrep")

            # Broadcast-DMA each [12] input to [4 partitions, 4, 12]:
            # element (p, c, q) <- input[q].  The free dim is c-major
            # (position c*12 + q) so the DMA's fastest-moving dim is the
            # contiguous 12-element DRAM read (DGE requires this).
            def bcast_in(dst, src):
                dst3 = dst[:].rearrange("p (c q) -> p c q", q=N_PAIRS)
                src3 = (
                    src.ap()
                    .unsqueeze(0)
                    .unsqueeze(1)
                    .broadcast_to((DIM, DIM, N_PAIRS))
                )
                nc.sync.dma_start(dst3, src3)

            bcast_in(kr, k_raw)
            bcast_in(irep, pairs_i)
            bcast_in(jrep, pairs_j)

            # col4[p, c*12 + q] = c ; row4[p, 0] = p
            col4 = pool.tile([DIM, W], I32, tag="col4")
            nc.gpsimd.iota(col4[:], pattern=[[1, DIM], [0, N_PAIRS]], base=0,
                           channel_multiplier=0)
            row4 = pool.tile([DIM, 1], F32, tag="row4")
            nc.gpsimd.iota(row4[:], pattern=[[0, 1]], base=0,
                           channel_multiplier=1,
                           allow_small_or_imprecise_dtypes=True)

            # Column one-hot masks: IM[p, 4q + c] = (c == i_q), JM likewise.
            im = pool.tile([DIM, W], F32, tag="im")
            nc.vector.tensor_tensor(im[:], col4[:], irep[:], op=ALU.is_equal)
            jm = pool.tile([DIM, W], F32, tag="jm")
            nc.vector.tensor_tensor(jm[:], col4[:], jrep[:], op=ALU.is_equal)

            # k_int = rne_round(3 * tanh(k_raw)), replicated [4, 48].
            ki = pool.tile([DIM, W], F32, tag="ki")
            nc.scalar.activation(ki[:], kr[:], AF.Tanh)
            nc.scalar.mul(ki[:], ki[:], 3.0)
            nc.vector.tensor_scalar(ki[:], ki[:], ROUND_MAGIC, None, op0=ALU.add)
            nc.vector.tensor_scalar(ki[:], ki[:], -ROUND_MAGIC, None, op0=ALU.add)

            # KJM[p, 4q + c] = k_int[q] * (c == j_q)
            kjm = pool.tile([DIM, W], F32, tag="kjm")
            nc.vector.tensor_tensor(kjm[:], jm[:], ki[:], op=ALU.mult)

            # Strided [4, 4] views of block p: columns {p, 12+p, 24+p, 36+p}.
            def blk(t, p):
                return t[:].rearrange("r (c q) -> r c q", q=N_PAIRS)[:, :, p]

            # R = I4 (blk(col4, 0) has value c in column c)
            r = pool.tile([DIM, DIM], F32, tag="r")
            nc.vector.tensor_scalar(r[:], blk(col4, 0), row4[:], None,
                                    op0=ALU.is_equal)

            scratch = pool.tile([DIM, DIM], F32, tag="scratch")
            u = pool.tile([DIM, 1], F32, tag="u")

            # R <- R @ S_p for p = 11..0  =>  R[:, j_p] += k_p * R[:, i_p]
            for p in reversed(range(N_PAIRS)):
                nc.vector.scalar_tensor_tensor(
                    out=scratch[:],
                    in0=r[:],
                    scalar=1.0,
                    in1=blk(im, p),
                    op0=ALU.mult,
                    op1=ALU.mult,
                    accum_out=u[:],
                )
                nc.vector.scalar_tensor_tensor(
                    out=r[:],
                    in0=blk(kjm, p),
                    scalar=u[:],
                    in1=r[:],
                    op0=ALU.mult,
                    op1=ALU.add,
                )

            nc.sync.dma_start(out.ap(), r[:])

    nc.compile()
    return nc


_NC_CACHE = None


def _get_module():
    global _NC_CACHE
    if _NC_CACHE is None:
        _NC_CACHE = _build_module()
    return _NC_CACHE


def kernel(k_raw, pairs_i, pairs_j):
    nc = _get_module()
    in_map = {
        "k_raw": np.ascontiguousarray(np.asarray(k_raw, dtype=np.float32)),
        "pairs_i": np.ascontiguousarray(np.asarray(pairs_i, dtype=np.int32)),
        "pairs_j": np.ascontiguousarray(np.asarray(pairs_j, dtype=np.int32)),
    }
    res = run_bass_kernel_spmd(nc, [in_map] * N_CORES, list(range(N_CORES)))
    return np.asarray(res.results[0]["out"], dtype=np.float32)
